# revision 48
# baseline (speedup 1.0000x reference)
"""Trainium2 Bass kernel for nn_MultiHeadAttention (B=4, S=2048, D=1024, H=16, DH=64).

Head-sharding: 8 cores = 4 batches x 2 head-groups (8 heads each). Each core
computes, for its (batch b, head-group g): Q/K/V projections of ITS heads
over the full sequence, masked softmax attention, and a PARTIAL output
projection (its heads' rows of Wo). The two partials per batch are summed on
the HOST in kernel()'s gather.

v2 schedule: the ScalarE exp is the bottleneck engine (~257us of LUT work at
1 elem/cycle), so the kernel is organized as one fused stream where the
attention loop feeds ACT continuously from the start and ALL projection
matmuls (V/K/Q/O) run as PE filler inside the ACT-bound attention rounds.

  sub-iteration = (q-block qb of 512, head-pair j); 16 total, qb-major.
  per sub-it: 7 kc-pairs; each round:
    scores: 4 MMs (hh0/hh1 adjacent at tile_position rows 0/64 -> they can
            overlap; kc-even/odd go to the two banks of ps_s[hh])
    exp:    2 ACT calls of [128,1024] (best ACT efficiency PSUM allows)
    PV:     4 MMs M=65 (V columns + mask column) accumulating ps_o[hh][65,512]
  Masking: host zeroes xK/xV columns at positions >= len and supplies the
  valid-mask as V's 65th column, so masked keys contribute exp(0)*0 = 0 to
  both numerator and l. No ACT bias read, no -inf handling.

PSUM budget (8 banks): ps_s 2x[128,1024]=4, ps_o 2x[65,512]=2, shared
projection accumulator [128,512]x2bufs=2.

Device layout is feature-major (512 hdh columns per core = 4 pairs):
  QT = Wq_g^T xqT / 8       [512, 2048]  (1/8 score scale + bq folded in)
  KT = Wk_g^T xkT           [512, S]     bf16, SBUF-resident
  V  = (Wv_g^T xvT)^T       [S, 512]     per head [s, h, 65], col 64 = mask
  yT_partial = Wo_g^T outT (+ bo' on g=0 only)   [D, 2048]
bk dropped (softmax-invariant); bv,bo fold into bo' = bv@Wo+bo host-side,
carried by g=0 alone so the host sum stays exact.
"""

import os
import sys
import numpy as np
import ml_dtypes

if "/opt/trn_rl_repo" not in sys.path:
    sys.path.insert(0, "/opt/trn_rl_repo")

import concourse.bass as bass
import concourse.mybir as mybir
import concourse.tile as tile
from concourse.tile import add_dep_helper
from concourse import bacc
from concourse.bass_utils import run_bass_kernel_spmd

B, S, D = 4, 2048, 1024
H, DH = 16, 64
HDH = H * DH                      # 1024
P = 128
DC = D // P                       # 8 contraction chunks
KC = S // P                       # 16 key chunks max
VW = DH + 1                       # 65: V columns per head + mask column
NJC = 4                           # head pairs per core (8 heads)
HW2 = NJC * P                     # 512 hdh columns per core
QB = 512                          # query block
NQB = S // QB                     # 4 q-blocks
F32 = mybir.dt.float32
BF16 = mybir.dt.bfloat16

_CACHE = {}


def build_bass(kc_lim=14, dbg=False):
    nc = bacc.Bacc("TRN2", target_bir_lowering=False, debug=False)
    klen = kc_lim * P
    n_kb = (klen + 511) // 512
    nkcp = (kc_lim + 1) // 2
    dbg_t = {}
    if dbg:
        dbg_t["qt"] = nc.dram_tensor("dbg_qt", [P, NJC * S], BF16,
                                     kind="ExternalOutput").ap()
        dbg_t["kt"] = nc.dram_tensor("dbg_kt", [P, NJC * klen], BF16,
                                     kind="ExternalOutput").ap()
        dbg_t["v"] = nc.dram_tensor("dbg_v", [P, kc_lim * (H // 2) * VW],
                                    BF16, kind="ExternalOutput").ap()
        dbg_t["et"] = nc.dram_tensor("dbg_et", [P, 2 * QB], BF16,
                                     kind="ExternalOutput").ap()
        dbg_t["cp"] = nc.dram_tensor("dbg_cp", [VW, QB], F32,
                                     kind="ExternalOutput").ap()
        dbg_t["xk"] = nc.dram_tensor("dbg_xk", [P, 4 * DC * 512], BF16,
                                     kind="ExternalOutput").ap()

    # Host supplies block-contiguous layouts: each DMA lands as one long
    # contiguous per-partition run (8KB) -> ~8x fewer DMA descriptors.
    # x*: [P, nblk, DC, 512] flattened; block b holds seq cols b*512..
    # w*: [P, DC*HW2]; wo: [P, NJC*D]
    xqT = nc.dram_tensor("xqT", [P, NQB * DC * QB], BF16,
                         kind="ExternalInput").ap()
    xkT = nc.dram_tensor("xkT", [P, 4 * DC * 512], BF16,
                         kind="ExternalInput").ap()
    xvT = nc.dram_tensor("xvT", [P, 4 * DC * 512], BF16,
                         kind="ExternalInput").ap()
    # wq/wk are j-major so the j=0 slices (critical path to the first
    # exp) can land in their own small DMAs
    wq = nc.dram_tensor("wq", [P, NJC * DC * P], BF16,
                        kind="ExternalInput").ap()
    wk = nc.dram_tensor("wk", [P, NJC * DC * P], BF16,
                        kind="ExternalInput").ap()
    wv = nc.dram_tensor("wv", [P, DC * HW2], BF16, kind="ExternalInput").ap()
    wo = nc.dram_tensor("wo", [P, NJC * D], BF16, kind="ExternalInput").ap()
    bq8 = nc.dram_tensor("bq8", [P, NJC], F32, kind="ExternalInput").ap()
    bo2 = nc.dram_tensor("bo2", [P, DC], F32, kind="ExternalInput").ap()
    maskr = nc.dram_tensor("maskr", [P, kc_lim * (H // 2)], BF16,
                           kind="ExternalInput").ap()
    yT = nc.dram_tensor("yT", [D, S], BF16, kind="ExternalOutput").ap()

    Exp = mybir.ActivationFunctionType.Exp
    AOp = mybir.AluOpType

    xq_bk = xqT.rearrange("p (b c s) -> p b c s", c=DC, s=QB)
    xk_bk = xkT.rearrange("p (b c s) -> p b c s", c=DC, s=512)
    xv_bk = xvT.rearrange("p (b c s) -> p b c s", c=DC, s=512)
    wq_bk = wq.rearrange("p (j c n) -> p j c n", c=DC, n=P)
    wk_bk = wk.rearrange("p (j c n) -> p j c n", c=DC, n=P)
    yt_ch = yT.rearrange("(c p) s -> c p s", p=P)

    with tile.TileContext(nc) as tc:
        with (
            tc.tile_pool(name="const", bufs=1) as cpool,
            tc.tile_pool(name="wts", bufs=1) as wpool,
            tc.tile_pool(name="xin", bufs=1) as xpool,
            tc.tile_pool(name="big", bufs=1) as bpool,
            tc.tile_pool(name="etp", bufs=1) as etp,
            tc.tile_pool(name="otp", bufs=1) as otp,
            tc.tile_pool(name="eps", bufs=1) as eps,
            tc.tile_pool(name="ytp", bufs=3) as ytp,
            tc.tile_pool(name="pss", bufs=1, space="PSUM") as pss,
            tc.tile_pool(name="pso", bufs=1, space="PSUM") as pso,
            tc.tile_pool(name="ppj", bufs=1, space="PSUM") as ppj,
            tc.tile_pool(name="rdram", bufs=4, space="DRAM") as rdp,
        ):
            # ---- constants -------------------------------------------
            bq8_sb = cpool.tile([P, NJC], F32)
            nc.scalar.dma_start(out=bq8_sb, in_=bq8)
            bo2_sb = cpool.tile([P, DC], F32)
            nc.scalar.dma_start(out=bo2_sb, in_=bo2)

            # ---- persistent SBUF tensors -----------------------------
            v_sb = bpool.tile([P, kc_lim, H // 2, VW], BF16, name="v")
            kt_sb = bpool.tile([P, NJC, klen], BF16, name="kt")
            qt_sb = bpool.tile([P, NJC, S], BF16, name="qt")

            # mask column of V (written once; V-proj fills cols 0:64)
            nc.sync.dma_start(
                out=v_sb[:, :, :, DH:VW],
                in_=maskr.rearrange("p (c h) -> p c h", h=H // 2),
            )

            # ---- weights ---------------------------------------------
            wk_sb = wpool.tile([P, NJC, DC, P], BF16, name="wk")
            wq_sb = wpool.tile([P, NJC, DC, P], BF16, name="wq")
            wv_sb = wpool.tile([P, DC, HW2], BF16, name="wv")
            wo_sb = wpool.tile([P, NJC, D], BF16, name="wo")
            # critical-path weights first: j=0 slices only; j=1..3 loaded
            # after the first x blocks below

            def load_wjk(j):
                nc.sync.dma_start(
                    out=wk_sb[:, j].rearrange("p a b -> p (a b)"),
                    in_=wk_bk[:, j].rearrange("p a b -> p (a b)"))
                nc.sync.dma_start(
                    out=wq_sb[:, j].rearrange("p a b -> p (a b)"),
                    in_=wq_bk[:, j].rearrange("p a b -> p (a b)"))

            load_wjk(0)
            nc.scalar.dma_start(
                out=wv_sb.rearrange("p a b -> p (a b)"), in_=wv)

            # ---- x inputs (gpsimd queue, consumption order) ----------
            xk_t = {}

            def load_xk(cb):
                t = xpool.tile([P, DC, 512], BF16, tag=f"xk{cb}",
                               name=f"xk{cb}")
                nc.gpsimd.dma_start(
                    out=t.rearrange("p a b -> p (a b)"),
                    in_=xk_bk[:, cb, :, :].rearrange("p a b -> p (a b)"))
                xk_t[cb] = t

            xv_t = {}
            n_vb = (kc_lim + 3) // 4

            def load_xv(cb):
                t = xpool.tile([P, DC, 512], BF16, tag="xv", bufs=2,
                               name="xv")
                nc.gpsimd.dma_start(
                    out=t.rearrange("p a b -> p (a b)"),
                    in_=xv_bk[:, cb, :, :].rearrange("p a b -> p (a b)"))
                xv_t[cb] = t

            xq_t = {}

            def load_xq(qb):
                t = xpool.tile([P, DC, QB], BF16, tag="xq", bufs=2,
                               name="xq")
                nc.scalar.dma_start(
                    out=t.rearrange("p a b -> p (a b)"),
                    in_=xq_bk[:, qb, :, :].rearrange("p a b -> p (a b)"))
                xq_t[qb] = t

            load_xk(0)
            load_xq(0)
            load_xv(0)
            load_xk(1)
            if n_vb > 1:
                load_xv(1)
            for cb in range(2, n_kb):
                load_xk(cb)
            load_xq(1)
            for j in range(1, NJC):
                load_wjk(j)
            # wo late on the gpsimd queue (needed first by qb0's O-proj)
            nc.gpsimd.dma_start(
                out=wo_sb.rearrange("p a b -> p (a b)"), in_=wo)

            # ---- projection group emitters (shared PSUM tag "pj") ----
            # ordering: projection matmuls emitted after a sub-it boundary
            # must not be scheduled ahead of that boundary's scores round
            after_bnd = [None]

            def ordep(mm):
                if after_bnd[0] is not None:
                    add_dep_helper(mm.ins, after_bnd[0], sync=False,
                                   reason="keep boundary scores ahead")

            def kproj_group(j, kb):
                w = min(512, klen - kb * 512)
                ps = ppj.tile([P, 512], F32, tag="pj", bufs=2)
                for kc in range(DC):
                    mm = nc.tensor.matmul(
                        ps[:, 0:w],
                        wk_sb[:, j, kc, :],
                        xk_t[kb][:, kc, 0:w],
                        start=(kc == 0), stop=(kc == DC - 1),
                    )
                    if kc == 0:
                        ordep(mm)
                nc.vector.tensor_copy(
                    kt_sb[:, j, kb * 512:kb * 512 + w], ps[:, 0:w])

            def qproj_group(j, qb):
                ps = ppj.tile([P, 512], F32, tag="pj", bufs=2)
                for kc in range(DC):
                    mm = nc.tensor.matmul(
                        ps,
                        wq_sb[:, j, kc, :],
                        xq_t[qb][:, kc, :],
                        start=(kc == 0), stop=(kc == DC - 1),
                    )
                    if kc == 0:
                        ordep(mm)
                nc.vector.tensor_scalar(
                    qt_sb[:, j, qb * QB:(qb + 1) * QB], ps,
                    0.125, bq8_sb[:, j:j + 1], AOp.mult, AOp.add)

            def vproj_group(sc):
                cb, scl = sc // 4, sc % 4
                ps = ppj.tile([P, 512], F32, tag="pj", bufs=2)
                for kc in range(DC):
                    mm = nc.tensor.matmul(
                        ps,
                        xv_t[cb][:, kc, scl * P:(scl + 1) * P],
                        wv_sb[:, kc, :],
                        start=(kc == 0), stop=(kc == DC - 1),
                    )
                    if kc == 0:
                        ordep(mm)
                nc.vector.tensor_copy(
                    v_sb[:, sc, :, 0:DH],
                    ps.rearrange("p (h d) -> p h d", d=DH))

            def oproj_group(dc, qb, ot_row):
                ps = ppj.tile([P, 512], F32, tag="pj", bufs=2)
                for j in range(NJC):
                    mm = nc.tensor.matmul(
                        ps,
                        wo_sb[:, j, dc * P:(dc + 1) * P],
                        ot_row[j],
                        start=(j == 0), stop=(j == NJC - 1),
                    )
                    if j == 0:
                        ordep(mm)
                yt_sb = ytp.tile([P, 512], BF16, tag="yt")
                nc.vector.tensor_scalar(
                    yt_sb, ps, bo2_sb[:, dc:dc + 1], None, AOp.add)
                eng = (nc.gpsimd, nc.sync, nc.scalar)[dc % 3]
                eng.dma_start(
                    out=yt_ch[dc][:, qb * QB:(qb + 1) * QB], in_=yt_sb)

            # last q-block: pre-accumulate j0..j2 so only the j3 matmul
            # and a DVE combine remain on the critical tail
            ycp = {}

            def oproj_groupA(dc, qb, ot_row):
                ps = ppj.tile([P, 512], F32, tag="pj", bufs=2)
                for j in range(NJC - 1):
                    mm = nc.tensor.matmul(
                        ps,
                        wo_sb[:, j, dc * P:(dc + 1) * P],
                        ot_row[j],
                        start=(j == 0), stop=(j == NJC - 2),
                    )
                    if j == 0:
                        ordep(mm)
                cp3 = ytp.tile([P, 512], F32, tag=f"ycp{dc}", bufs=1,
                               name="ycp")
                nc.vector.tensor_copy(cp3, ps)
                ycp[dc] = cp3

            def oproj_groupB(dc, qb, ot3):
                ps = ppj.tile([P, 512], F32, tag="pj", bufs=2)
                nc.tensor.matmul(
                    ps, wo_sb[:, NJC - 1, dc * P:(dc + 1) * P], ot3,
                )
                yt_sb = ytp.tile([P, 512], BF16, tag="yt")
                nc.vector.scalar_tensor_tensor(
                    yt_sb, ps, bo2_sb[:, dc:dc + 1], ycp.pop(dc),
                    AOp.add, AOp.add)
                eng = (nc.gpsimd, nc.sync, nc.scalar)[dc % 3]
                eng.dma_start(
                    out=yt_ch[dc][:, qb * QB:(qb + 1) * QB], in_=yt_sb)

            # ---- filler schedule -------------------------------------
            # fillers[si] = list of closures to emit inside sub-it si
            nsub = NQB * NJC
            fillers = [[] for _ in range(nsub + 1)]

            def F(si, fn, *a):
                fillers[min(si, nsub)].append((fn,) + a)

            NOFILL = bool(os.environ.get("NOFILL"))
            if not NOFILL:
                # V-proj sc2.. -> consumed by sub-it0 PV rounds (handled
                # inline below); K j1..3 and Q are spread with deadlines:
                for j in range(1, NJC):
                    for kb in range(n_kb):
                        F(j - 1, kproj_group, j, kb)
                    F(j - 1, qproj_group, j, 0)
                qi = 3
                for qb in range(1, NQB):
                    for j in range(NJC):
                        F(qi, qproj_group, j, qb)
                        qi += 1
                    if qb + 1 < NQB:
                        F(qi - 2, load_xq, qb + 1)

            # ---- prologue projections --------------------------------
            for kb in range(n_kb):
                kproj_group(0, kb)
            qproj_group(0, 0)
            vproj_group(0)
            vproj_group(1)
            if NOFILL:
                for j in range(1, NJC):
                    for kb in range(n_kb):
                        kproj_group(j, kb)
                for sc in range(2, kc_lim):
                    if sc % 4 == 0 and sc // 4 + 1 < n_vb:
                        load_xv(sc // 4 + 1)
                    vproj_group(sc)
                for qb in range(NQB):
                    for j in range(NJC):
                        if (j, qb) != (0, 0):
                            qproj_group(j, qb)
                    if qb + 1 < NQB:
                        load_xq(qb + 1)

            # ---- attention -------------------------------------------
            ets = {}

            def scores_round(si2, kcp):
                # 4 MMs: (hh0,even),(hh1,even),(hh0,odd),(hh1,odd)
                qb2, j2 = si2 // NJC, si2 % NJC
                q0 = qb2 * QB
                ps_pair = [pss.tile([P, 2 * QB], F32, tag=f"s{hh}",
                                    name="ps_s") for hh in range(2)]
                pars = [0, 1] if 2 * kcp + 1 < kc_lim else [0]
                last_mm = None
                for par in pars:
                    kc = 2 * kcp + par
                    for hh in range(2):
                        last_mm = nc.tensor.matmul(
                            ps_pair[hh][:, par * QB:(par + 1) * QB],
                            kt_sb[hh * DH:(hh + 1) * DH, j2,
                                  kc * P:(kc + 1) * P],
                            qt_sb[hh * DH:(hh + 1) * DH, j2, q0:q0 + QB],
                            tile_position=(hh * DH, 0),
                        )
                nw = len(pars) * QB
                for hh in range(2):
                    et = etp.tile([P, 2 * QB], BF16, tag=f"e{hh}", bufs=4,
                                  name="et")
                    nc.scalar.activation(
                        et[:, 0:nw], ps_pair[hh][:, 0:nw], Exp,
                        bias=0.0, scale=1.0)
                    ets[(si2, kcp, hh)] = et
                    if dbg and si2 == 0 and kcp == 0 and hh == 0:
                        nc.gpsimd.dma_start(out=dbg_t["et"], in_=et)
                return last_mm

            def pv_round(si2, kcp, ps_o):
                j2 = si2 % NJC
                pars = [0, 1] if 2 * kcp + 1 < kc_lim else [0]
                for par in pars:
                    kc = 2 * kcp + par
                    for hh in range(2):
                        et = ets[(si2, kcp, hh)]
                        nc.tensor.matmul(
                            ps_o[hh],
                            v_sb[:, kc, 2 * j2 + hh, :],
                            et[:, par * QB:(par + 1) * QB],
                            start=(kc == 0), stop=(kc == kc_lim - 1),
                        )
                for hh in range(2):
                    del ets[(si2, kcp, hh)]

            ot_tiles = {}
            scores_round(0, 0)
            for si in range(nsub):
                qb, j = si // NJC, si % NJC
                ps_o = [pso.tile([VW, QB], F32, tag=f"o{hh}", name="ps_o")
                        for hh in range(2)]
                ot = otp.tile([P, QB], BF16, tag=f"ot{j}", bufs=2,
                              name=f"ot{j}")
                ot_tiles[(j, qb)] = ot
                fq = list(fillers[si])
                fi = 0
                for kcp in range(nkcp):
                    # sub-it 0: keep V-proj just ahead of PV consumption
                    if si == 0 and not NOFILL:
                        for sc in (2 * kcp + 2, 2 * kcp + 3):
                            if sc < kc_lim:
                                if sc % 4 == 0 and sc // 4 + 1 < n_vb:
                                    load_xv(sc // 4 + 1)
                                vproj_group(sc)
                    # 1-round software pipeline: emit the NEXT scores round
                    # (crossing into sub-it si+1 at the boundary) so ACT is
                    # never starved behind epilogue/O-proj filler work.
                    if kcp + 1 < nkcp:
                        scores_round(si, kcp + 1)
                    elif si + 1 < nsub:
                        # boundary: make the next sub-it's first scores the
                        # PE's top pick the moment the ps_s slots free, so
                        # ACT isn't starved behind epilogue/O-proj fillers
                        with tc.high_priority():
                            after_bnd[0] = scores_round(si + 1, 0).ins
                    if fi < len(fq):
                        fn = fq[fi]
                        fn[0](*fn[1:])
                        fi += 1
                    pv_round(si, kcp, ps_o)
                while fi < len(fq):
                    fn = fq[fi]
                    fn[0](*fn[1:])
                    fi += 1

                # ---- epilogue: ot = ps_o[0:64] / l ------------------
                for hh in range(2):
                    cp = eps.tile([VW, QB], F32, tag=f"cp{hh}")
                    nc.vector.tensor_copy(cp, ps_o[hh])
                    if dbg and si == 0 and hh == 0:
                        nc.gpsimd.dma_start(out=dbg_t["cp"], in_=cp)
                    rd = rdp.tile([1, QB], F32, tag="rd", name="rd")
                    nc.sync.dma_start(out=rd, in_=cp[DH:VW, :])
                    rd_b = bass.AP(tensor=rd.tensor, offset=rd.offset,
                                   ap=[[0, DH], rd.ap[-1]])
                    L = eps.tile([DH, QB], F32, tag=f"L{hh}")
                    nc.sync.dma_start(out=L, in_=rd_b)
                    nc.vector.reciprocal_approx_fast(L, L)
                    if hh == 0:
                        nc.vector.tensor_mul(ot[0:DH, :], cp[0:DH, :], L)
                    else:
                        tmpB = eps.tile([DH, QB], BF16, tag="tmpB")
                        nc.vector.tensor_mul(tmpB, cp[0:DH, :], L)
                        nc.gpsimd.dma_start(out=ot[DH:P, :], in_=tmpB)

                # ---- O-proj for completed q-block -------------------
                if j == NJC - 1:
                    if qb == NQB - 1:
                        ot3 = ot_tiles.pop((NJC - 1, qb))
                        for dc in range(DC):
                            oproj_groupB(dc, qb, ot3)
                    else:
                        ot_row = [ot_tiles.pop((jj, qb))
                                  for jj in range(NJC)]
                        for dc in range(DC):
                            oproj_group(dc, qb, ot_row)
                elif j == NJC - 2 and qb == NQB - 1:
                    # pre-accumulate the last q-block's j0..j2 O-proj
                    # partials while sub-it 15 runs
                    rowA = [ot_tiles[(jj, qb)] for jj in range(NJC - 1)]
                    for dc in range(DC):
                        oproj_groupA(dc, qb, rowA)
                    for jj in range(NJC - 1):
                        del ot_tiles[(jj, qb)]

            if dbg:
                for cb in range(n_kb):
                    nc.sync.dma_start(
                        out=dbg_t["xk"][:, cb * DC * 512:(cb + 1) * DC * 512],
                        in_=xk_t[cb].rearrange("p a b -> p (a b)"))
                nc.sync.dma_start(
                    out=dbg_t["qt"],
                    in_=qt_sb.rearrange("p a b -> p (a b)"))
                nc.sync.dma_start(
                    out=dbg_t["kt"],
                    in_=kt_sb.rearrange("p a b -> p (a b)"))
                nc.sync.dma_start(
                    out=dbg_t["v"],
                    in_=v_sb.rearrange("p a b c -> p (a b c)"))

    nc.compile()
    return nc


def _prepare(x_Q, x_K, x_V, src_batch_lens, Wq, bq, Wk, bk, Wv, bv, Wo, bo):
    bf16 = ml_dtypes.bfloat16
    x_Q = np.asarray(x_Q, dtype=np.float32)
    x_K = np.asarray(x_K, dtype=np.float32)
    x_V = np.asarray(x_V, dtype=np.float32)
    lens = np.asarray(src_batch_lens)
    Wq = np.ascontiguousarray(np.asarray(Wq, dtype=np.float32))
    Wk = np.ascontiguousarray(np.asarray(Wk, dtype=np.float32))
    Wv = np.ascontiguousarray(np.asarray(Wv, dtype=np.float32))
    Wo = np.ascontiguousarray(np.asarray(Wo, dtype=np.float32))
    bq = np.asarray(bq, dtype=np.float32)
    bv = np.asarray(bv, dtype=np.float32)
    bo = np.asarray(bo, dtype=np.float32)

    maxlen = max(1, min(S, int(np.max(lens))))
    kc_lim = (maxlen + P - 1) // P
    klen = kc_lim * P

    # bo' = bv@Wo + bo is exact only if added ONCE: g=0 carries it, g=1 zeros
    bo2_full = (bv @ Wo + bo).astype(np.float32)
    bo2_g = [np.ascontiguousarray(bo2_full.reshape(DC, P).T),
             np.zeros((P, DC), np.float32)]

    def xblocks(x):
        # [S, D] -> [P, 4, DC, 512]: block b = seq cols b*512.., each
        # partition's run contiguous (device tile layout [P, DC, 512])
        return np.ascontiguousarray(
            x.reshape(4, 512, DC, P).transpose(3, 0, 2, 1)
        ).reshape(P, 4 * DC * 512)

    def wblocks(w):
        # [D, 512] -> [P, DC*512]
        return np.ascontiguousarray(
            w.reshape(DC, P, HW2).transpose(1, 0, 2)).reshape(P, DC * HW2)

    def wjmajor(w):
        # [D, 512] -> [P, NJC*DC*128]: j outer so per-j slices are
        # contiguous single DMAs
        return np.ascontiguousarray(
            w.reshape(DC, P, NJC, P).transpose(1, 2, 0, 3)
        ).reshape(P, NJC * DC * P)

    k_idx = np.arange(S)
    in_maps = []
    for c in range(8):
        b, g = c // 2, c % 2
        ln = int(lens[b])
        hs = slice(g * HW2, (g + 1) * HW2)
        # zero masked key rows of xK/xV: masked keys then contribute
        # exp(0)*0 = 0 to both the PV numerator and (via the mask column
        # of V) the softmax denominator.
        xkb = x_K[b].copy()
        xkb[ln:] = 0.0
        xvb = x_V[b].copy()
        xvb[ln:] = 0.0
        mvalid = (k_idx[:klen] < ln).astype(np.float32)  # [klen]
        # maskr[p, (sc, h)] = valid(sc*128 + p), repeated over 8 head slots
        maskr = np.repeat(
            mvalid.reshape(kc_lim, P).T[:, :, None], H // 2, axis=2
        ).reshape(P, kc_lim * (H // 2))
        in_maps.append({
            "xqT": xblocks(x_Q[b]).astype(bf16),
            "xkT": xblocks(xkb).astype(bf16),
            "xvT": xblocks(xvb).astype(bf16),
            "wq": wjmajor(Wq[:, hs]).astype(bf16),
            "wk": wjmajor(Wk[:, hs]).astype(bf16),
            "wv": wblocks(Wv[:, hs]).astype(bf16),
            "wo": np.ascontiguousarray(
                Wo[hs, :].reshape(NJC, P, D).transpose(1, 0, 2)
            ).reshape(P, NJC * D).astype(bf16),
            "bq8": np.ascontiguousarray(
                (bq[hs] / 8.0).reshape(NJC, P).T),
            "bo2": bo2_g[g],
            "maskr": np.ascontiguousarray(maskr).astype(bf16),
        })
    return kc_lim, in_maps


def _build_in_maps(inputs):
    return _prepare(**inputs)[1]


def kernel(x_Q, x_K, x_V, src_batch_lens, Wq, bq, Wk, bk, Wv, bv, Wo, bo):
    kc_lim, in_maps = _prepare(x_Q, x_K, x_V, src_batch_lens,
                               Wq, bq, Wk, bk, Wv, bv, Wo, bo)
    if kc_lim not in _CACHE:
        _CACHE[kc_lim] = build_bass(kc_lim)
    nc = _CACHE[kc_lim]

    res = run_bass_kernel_spmd(nc, in_maps, core_ids=list(range(8)))

    out = np.empty((B, S, D), dtype=np.float32)
    for b in range(B):
        out[b] = (res.results[2 * b]["yT"].astype(np.float32) +
                  res.results[2 * b + 1]["yT"].astype(np.float32)).T
    return out
